# revision 13
# baseline (speedup 1.0000x reference)
"""Trainium2 Bass kernel for nn_AttentionLayer (pooling attention).

Computes, for each batch b and head i:
    own  = inputs[b,i,:] @ W1_own[i] + b1[i]          # [64]
    ev   = inputs[b,j,:] @ W1_ev[i]                   # [j,64]
    h    = relu(own + ev)                             # [j,64]
    s    = h @ W2[i]                                  # [j]
    w    = softmax_j(s)
    out[b,i] = sum_j w[j] * inputs[b,j]

Key identity: max(ev, -(own+b1)) = relu(ev+own+b1) - (own+b1); the
correction is constant in j, so softmax is unchanged — no separate
relu pass needed.

v4:
  * ev contraction is hybrid fp8/bf16: d-tiles 0..3 run as DoubleRow
    fp8 matmuls (K=256 each), d-tiles 4..5 stay bf16.  W1e/W1o/b1 are
    pre-scaled by 1024 (lossless power of two) so fp8 weights sit in
    e4m3's +-240 range; W2 is scaled by 1/1024 so softmax temperature
    is unchanged.  Measured numerically: rel err ~1.43e-2 < 2e-2.
  * DoubleRow LDWEIGHTS cannot overlap an in-flight DR matmul, so
    chunks are processed in PAIRS sharing each weight load (the last
    two chunks stay single so the softmax/pool drain tail is short).
  * scores use 4x column tiling (128x32 PE mode); softmax runs on the
    128-partition scattered layout; a selection-matrix PE transpose
    gathers the 16 score rows back contiguous for pooling.
  * own (j-major xt2 layout, N=512 matmuls) rides chunk pair 0 with a
    one-tile lag so its weight/input DMAs never stall the PE queue.
  * startup DMAs are sliced fine (weights per k-part x t-pair, first
    chunk per k-part) and ordered by first consumption.

Sharding: data-parallel over batch across 8 NeuronCores (256
batches/core).  All parameters replicated; no collectives.

Self-contained: hardcodes shapes; only needs /opt/trn_rl_repo on
sys.path.
"""

import os
import sys
from contextlib import ExitStack

import numpy as np

if "/opt/trn_rl_repo" not in sys.path:
    sys.path.insert(0, "/opt/trn_rl_repo")
os.environ.setdefault("MYCRO_LOCAL_CACHE", "1")

import ml_dtypes  # noqa: E402

import concourse.bass as bass  # noqa: E402
import concourse.mybir as mybir  # noqa: E402
import concourse.tile as tile  # noqa: E402
from concourse import bacc  # noqa: E402
from concourse import bass_utils  # noqa: E402

# Problem shapes (hardcoded per spec)
B, NINS, D, H = 2048, 16, 768, 64
NCORES = 8
BC = B // NCORES          # 256 batches per core
R = BC * NINS             # 4096 rows (b,j) per core
KT = D // 128             # 6 contraction k-tiles
KP8 = 2                   # fp8 DoubleRow k-pairs (d-tiles 0..3)
KB16 = KT - 2 * KP8       # trailing bf16 k-tiles (d-tiles 4..5)
MT = NINS // 2            # 8 m-tiles of (il,h): tile t holds heads 2t, 2t+1
NCH = 8                   # column chunks per core
CHUNK = R // NCH          # 512 (b,j) columns per chunk
CB = CHUNK // NINS        # 32 batches per chunk
NBLK = R // 128           # 32 row-blocks per core
WSCALE = 1024.0           # power-of-two pre-scale for W1/b1 (W2 /= it)

BF = mybir.dt.bfloat16
F32 = mybir.dt.float32
FP8 = mybir.dt.float8e4
BF_NP = ml_dtypes.bfloat16
FP8_NP = ml_dtypes.float8_e4m3

_CACHED_NC = None
LAST_RESULTS = None


def build_nc():
    nc = bacc.Bacc("TRN2", target_bir_lowering=False, debug=False,
                   num_devices=NCORES)

    # b-major transposes of x for the ev matmuls, chunk-major.
    # fp8 part: d-tiles 0..3 interleaved (kp, ko) for DoubleRow.
    xt8_d = nc.dram_tensor("xt8", [128, NCH, KP8, 2, CHUNK], FP8,
                           kind="ExternalInput").ap()
    xtb_d = nc.dram_tensor("xtb", [128, NCH, KB16, CHUNK], BF,
                           kind="ExternalInput").ap()
    # j-major transpose for own
    xt2_d = nc.dram_tensor("xt2", [128, MT, KT, CHUNK], BF,
                           kind="ExternalInput").ap()
    xn_d = nc.dram_tensor("xn", [128, NBLK, D], BF,
                          kind="ExternalInput").ap()
    w1e8_d = nc.dram_tensor("w1e8", [128, KP8, 2, NINS * H], FP8,
                            kind="ExternalInput").ap()
    w1eb_d = nc.dram_tensor("w1eb", [128, KB16, NINS * H], BF,
                            kind="ExternalInput").ap()
    w1o_d = nc.dram_tensor("w1o", [128, KT, NINS * H], BF,
                           kind="ExternalInput").ap()
    w2t_d = nc.dram_tensor("w2t", [128, MT, 32], BF,
                           kind="ExternalInput").ap()
    b1n_d = nc.dram_tensor("b1n", [128, MT], F32, kind="ExternalInput").ap()
    msk_d = nc.dram_tensor("msk", [128, 128], BF, kind="ExternalInput").ap()
    sel_d = nc.dram_tensor("sel", [128, 16], BF, kind="ExternalInput").ap()
    out_d = nc.dram_tensor("out", [128, NBLK, D], BF,
                           kind="ExternalOutput").ap()

    with tile.TileContext(nc) as tc, ExitStack() as ctx:
        const = ctx.enter_context(tc.tile_pool(name="const", bufs=1))
        xtp = ctx.enter_context(tc.tile_pool(name="xtp", bufs=4))
        xt2p = ctx.enter_context(tc.tile_pool(name="xt2p", bufs=4))
        xnp = ctx.enter_context(tc.tile_pool(name="xnp", bufs=6))
        ownsb = ctx.enter_context(tc.tile_pool(name="ownsb", bufs=1))
        hp = ctx.enter_context(tc.tile_pool(name="hp", bufs=18))
        sm = ctx.enter_context(tc.tile_pool(name="sm", bufs=2))
        bdp = ctx.enter_context(tc.tile_pool(name="bdp", bufs=3))
        outp = ctx.enter_context(tc.tile_pool(name="outp", bufs=4))
        # PSUM (8 banks): ev(+wgtT borrow) 4 + scores 2 + pool/own 2.
        # own runs only during pair 0, pooling only starts after pair 0,
        # so they share the plps ring.
        evps = ctx.enter_context(tc.tile_pool(name="evps", bufs=4,
                                              space="PSUM"))
        scps = ctx.enter_context(tc.tile_pool(name="scps", bufs=2,
                                              space="PSUM"))
        plps = ctx.enter_context(tc.tile_pool(name="plps", bufs=2,
                                              space="PSUM"))

        # --- constants.  scalar ring: ev weights (sliced fine, in
        # consumption order) + xt2 + small consts; gpsimd ring: own
        # weights (outputs only start a few chunks in).
        b1n_sb = const.tile([128, MT], F32, tag="b1n")
        nc.scalar.dma_start(b1n_sb[:], b1n_d[:])
        w1e8_sb = const.tile([128, KP8, 2, NINS * H], FP8, tag="w1e8")
        w1eb_sb = const.tile([128, KB16, NINS * H], BF, tag="w1eb")
        w1o_sb = const.tile([128, KT, NINS * H], BF, tag="w1o")
        for k in range(KT):
            nc.gpsimd.dma_start(w1o_sb[:, k, :], w1o_d[:, k, :])
        w2t_sb = const.tile([128, MT, 32], BF, tag="w2t")
        msk_sb = const.tile([128, 128], BF, tag="msk")
        sel_sb = const.tile([128, 16], BF, tag="sel")

        def dma_wslices(tq):
            # one t-pair (256 cols) of every k-part, consumption order
            sl = slice(tq * 256, (tq + 1) * 256)
            for kp in range(KP8):
                nc.scalar.dma_start(w1e8_sb[:, kp, :, sl],
                                    w1e8_d[:, kp, :, sl])
            for kb in range(KB16):
                nc.scalar.dma_start(w1eb_sb[:, kb, sl], w1eb_d[:, kb, sl])

        xt_tiles = {}   # c -> (fp8 tile, bf16 tile)
        xt2_tiles = {}
        xn_tiles = {}

        def dma_xt(c, split=False):
            if c >= NCH:
                return
            t8 = xtp.tile([128, KP8, 2, CHUNK], FP8, tag="xt8", name="xt8t")
            tb = xtp.tile([128, KB16, CHUNK], BF, tag="xtb", name="xtbt")
            if split:
                for kp in range(KP8):
                    nc.sync.dma_start(t8[:, kp], xt8_d[:, c, kp])
                for kb in range(KB16):
                    nc.sync.dma_start(tb[:, kb], xtb_d[:, c, kb])
            else:
                nc.sync.dma_start(t8[:], xt8_d[:, c])
                nc.sync.dma_start(tb[:], xtb_d[:, c])
            xt_tiles[c] = (t8, tb)

        def dma_xt2(t, ks=None):
            if t >= MT:
                return
            if t not in xt2_tiles:
                xt2_tiles[t] = xt2p.tile([128, KT, CHUNK], BF, tag="xt2",
                                         name="xt2t")
            t_ = xt2_tiles[t]
            if ks is None:
                nc.scalar.dma_start(t_[:], xt2_d[:, t])
            else:
                for k in ks:
                    nc.scalar.dma_start(t_[:, k, :], xt2_d[:, t, k, :])

        def dma_xn(c):
            if c >= NCH:
                return
            t_ = xnp.tile([128, 4, D], BF, tag="xn", name="xnt")
            nc.sync.dma_start(t_[:], xn_d[:, c * 4:(c + 1) * 4, :])
            xn_tiles[c] = t_

        # startup DMA order (first consumed first per ring)
        dma_xt(0, split=True)
        dma_wslices(0)
        dma_xt2(0, ks=[0, 1])
        dma_xt(1, split=True)
        dma_wslices(1)
        dma_xt2(0, ks=[2, 3])
        dma_wslices(2)
        dma_xt2(0, ks=[4, 5])
        dma_wslices(3)
        dma_xt(2)
        dma_xt2(1)
        nc.scalar.dma_start(w2t_sb[:], w2t_d[:])
        nc.scalar.dma_start(sel_sb[:], sel_d[:])
        dma_xt(3)
        dma_xt2(2)
        nc.scalar.dma_start(msk_sb[:], msk_d[:])
        dma_xt2(3)
        dma_xn(0)
        dma_xn(1)

        # --- own: ownneg128[(il,h), t, b] = -(own'[b,2t+il,h] + b1'[2t+il,h])
        # (primes = x WSCALE).  One N=512 matmul per (t, k) on the
        # j-major layout; interleaved into pair 0's ev stream, one
        # tile behind so its DMAs never stall the PE queue.
        own128 = ownsb.tile([128, MT, BC], BF, tag="own")
        own_ps = {}

        def own_mm(t, k):
            if t < 0 or t >= MT:
                return
            if k == 0:
                own_ps[t] = plps.tile([128, 2, BC], F32, tag="pp",
                                      name="ownp")
            nc.tensor.matmul(
                own_ps[t][:], lhsT=w1o_sb[:, k, t * 128:(t + 1) * 128],
                rhs=xt2_tiles[t][:, k, :],
                start=(k == 0), stop=(k == KT - 1),
                skip_group_check=True,
            )

        def own_retire(t):
            if t >= MT:
                return
            ops = own_ps.pop(t)
            for il in range(2):
                nc.vector.scalar_tensor_tensor(
                    own128[il * H:(il + 1) * H, t, :],
                    ops[il * H:(il + 1) * H, il, :], -1.0,
                    b1n_sb[il * H:(il + 1) * H, t, None]
                    .to_broadcast([H, BC]),
                    mybir.AluOpType.mult, mybir.AluOpType.add)
            dma_xt2(t + 4)

        def do_softmax(scp):
            # scores are O(3); safe to exp without max subtraction.
            # only 16 of 128 rows hold scores; the rest are exact zeros
            # (the col-tiled matmuls write zeros there) so everything
            # stays finite and the sel-transpose drops them.
            ex = sm.tile([128, CB, NINS], F32, tag="ex")
            nc.scalar.activation(ex[:],
                                 scp.rearrange("p (b j) -> p b j", j=NINS),
                                 mybir.ActivationFunctionType.Exp)
            ssum = sm.tile([128, CB], F32, tag="ssum")
            nc.vector.tensor_reduce(ssum[:], ex[:], axis=mybir.AxisListType.X,
                                    op=mybir.AluOpType.add)
            rinv = sm.tile([128, CB], F32, tag="rinv")
            nc.vector.reciprocal(rinv[:], ssum[:])
            wgt = sm.tile([128, CHUNK], BF, tag="wgt")
            nc.vector.tensor_tensor(
                wgt.rearrange("p (b j) -> p b j", j=NINS),
                ex[:], rinv[:, :, None].to_broadcast([128, CB, NINS]),
                mybir.AluOpType.mult)
            return wgt

        def emit_wgtT(wgt):
            # borrow one evps ring buffer; bitcast a bf16 view for the
            # transpose outputs ([128, 4, 16] bf16 = 128 f32 bytes).
            # sel gathers the 16 scattered score rows -> contiguous.
            tpf = evps.tile([128, CHUNK], F32, tag="ev")
            tp = tpf[:, :32].bitcast(BF).rearrange("p (r i) -> p r i", i=NINS)
            for rt in range(4):
                nc.tensor.transpose(tp[:, rt, :],
                                    wgt[:, rt * 128:(rt + 1) * 128],
                                    sel_sb[:])
            return tp

        def emit_pool_rt(c, tp, rt):
            bd = bdp.tile([128, 8, NINS], BF, tag="bd")
            nc.vector.tensor_tensor(
                bd[:], tp[:, rt, None, :].to_broadcast([128, 8, NINS]),
                msk_sb.rearrange("p (g i) -> p g i", i=NINS),
                mybir.AluOpType.mult)
            bdf = bd.rearrange("p g i -> p (g i)")
            pp0 = plps.tile([128, 384], F32, tag="pp")
            pp1 = plps.tile([128, 384], F32, tag="pp")
            nc.tensor.matmul(pp0[:], lhsT=bdf, rhs=xn_tiles[c][:, rt, :384],
                             start=True, stop=True, skip_group_check=True)
            nc.tensor.matmul(pp1[:], lhsT=bdf, rhs=xn_tiles[c][:, rt, 384:],
                             start=True, stop=True, skip_group_check=True)
            return pp0, pp1

        def emit_out_rt(c, rt, pp0, pp1):
            osb = outp.tile([128, D], BF, tag="osb")
            nc.scalar.copy(osb[:, :384], pp0[:])
            nc.scalar.copy(osb[:, 384:], pp1[:])
            nc.gpsimd.dma_start(out_d[:, c * 4 + rt, :], osb[:])

        pqueue = []  # chunks awaiting pooling: {c, wgt, tp, piece}

        def pool_advance():
            """Emit the next pooling piece (wgtT, then 4 rt blocks)."""
            if not pqueue:
                return
            st = pqueue[0]
            if st["piece"] == 0:
                st["tp"] = emit_wgtT(st["wgt"])
            else:
                rt = st["piece"] - 1
                ppa, ppb = emit_pool_rt(st["c"], st["tp"], rt)
                emit_out_rt(st["c"], rt, ppa, ppb)
            st["piece"] += 1
            if st["piece"] == 5:
                pqueue.pop(0)

        def emit_scores(hts):
            scp = scps.tile([128, CHUNK], F32, tag="scp")
            for t in range(MT):
                v, ct = t // 4, t % 4
                nc.tensor.matmul(
                    scp[32 * ct:32 * (ct + 1), :],
                    lhsT=w2t_sb[:, t, :], rhs=hts[t],
                    start=(v == 0), stop=(v == 1),
                    tile_position=(0, 32 * ct),
                    skip_group_check=True,
                )
            return scp

        def ev_mms(cs, t, evs, with_own):
            """ev for tile t of every chunk in cs (shared weight loads);
            pair 0 interleaves own tile t-1 (2:3) between the groups."""
            ts = [xt_tiles[c] for c in cs]
            ok = iter(range(KT))
            for kp in range(KP8):
                for ci, c in enumerate(cs):
                    nc.tensor.matmul(
                        evs[ci][:],
                        lhsT=w1e8_sb[:, kp, :, t * 128:(t + 1) * 128],
                        rhs=ts[ci][0][:, kp, :, :],
                        start=(kp == 0), stop=False,
                        perf_mode=mybir.MatmulPerfMode.DoubleRow,
                        skip_group_check=True,
                    )
                if with_own:
                    own_mm(t - 1, next(ok))
                    own_mm(t - 1, next(ok))
            for kb in range(KB16):
                for ci, c in enumerate(cs):
                    nc.tensor.matmul(
                        evs[ci][:],
                        lhsT=w1eb_sb[:, kb, t * 128:(t + 1) * 128],
                        rhs=ts[ci][1][:, kb, :],
                        start=False, stop=(kb == KB16 - 1),
                        skip_group_check=True,
                    )
                if with_own:
                    own_mm(t - 1, next(ok))

        def emit_h(c, t, evp):
            h_t = hp.tile([128, CB, NINS], BF, tag="h")
            nc.vector.tensor_tensor(
                h_t[:], evp.rearrange("p (b j) -> p b j", j=NINS),
                own128[:, t, c * CB:(c + 1) * CB, None]
                .to_broadcast([128, CB, NINS]),
                mybir.AluOpType.max)
            return h_t.rearrange("p b j -> p (b j)")

        def emit_group(cs, with_own=False):
            """Process chunks cs (1 or 2) through ev/h/scores/softmax,
            while draining the pooling of previous chunks."""
            hts = {c: [] for c in cs}
            lag = 1 if with_own else 0
            evps_t = {}
            for t in range(MT + lag):
                if t < MT:
                    evps_t[t] = [evps.tile([128, CHUNK], F32, tag="ev",
                                           name="evt")
                                 for _ in cs]
                    ev_mms(cs, t, evps_t[t], with_own)
                elif with_own:
                    # own tile 7 has no ev matmuls left to ride on
                    for k in range(KT):
                        own_mm(MT - 1, k)
                th = t - lag
                if 0 <= th < MT:
                    if with_own:
                        own_retire(th)
                    for ci, c in enumerate(cs):
                        hts[c].append(emit_h(c, th, evps_t[th][ci]))
                        pool_advance()
                    del evps_t[th]
                if t == 4:
                    for c in cs:
                        dma_xn(c + 2)
            # drain pooling not covered by the h slots before scores
            while pqueue:
                pool_advance()
            scps_c = [emit_scores(hts[c]) for c in cs]
            for c in cs:
                dma_xt(c + 4)
            for ci, c in enumerate(cs):
                pqueue.append({"c": c, "wgt": do_softmax(scps_c[ci]),
                               "tp": None, "piece": 0})

        emit_group([0, 1], with_own=True)
        emit_group([2, 3])
        emit_group([4, 5])
        emit_group([6])
        emit_group([7])

        # drain the last chunk's pooling
        while pqueue:
            pool_advance()

    nc.compile()
    return nc


def host_prep(W1, b1, W2):
    """Build the replicated parameter tensors (numpy)."""
    W1 = np.asarray(W1, dtype=np.float32)
    b1 = np.asarray(b1, dtype=np.float32)
    W2 = np.asarray(W2, dtype=np.float32)
    W1o, W1e = W1[:, :D, :] * WSCALE, W1[:, D:, :] * WSCALE

    def to_cols(w):  # [16, 768, 64] -> [768, 1024] (cols i*64+h)
        return np.ascontiguousarray(
            w.transpose(1, 0, 2).reshape(D, NINS * H))

    we = to_cols(W1e)   # [768, 1024], pre-scaled
    # fp8 DoubleRow part: d-tiles 0..3 -> [128, KP8, 2, 1024]
    w1e8 = np.ascontiguousarray(
        we[:512].reshape(KP8, 2, 128, NINS * H)
        .transpose(2, 0, 1, 3)).astype(FP8_NP)
    # bf16 part: d-tiles 4..5 -> [128, KB16, 1024]
    w1eb = np.ascontiguousarray(
        we[512:].reshape(KB16, 128, NINS * H)
        .transpose(1, 0, 2)).astype(BF_NP)
    w1o = np.ascontiguousarray(
        to_cols(W1o).reshape(KT, 128, NINS * H)
        .transpose(1, 0, 2)).astype(BF_NP)

    w2t = np.zeros((128, MT, 32), dtype=np.float32)
    b1n = np.zeros((128, MT), dtype=np.float32)
    sel = np.zeros((128, 16), dtype=np.float32)
    for t in range(MT):
        v, c = t // 4, t % 4
        for il in range(2):
            i = 2 * t + il
            w2t[il * H:(il + 1) * H, t, 2 * v + il] = W2[i] / WSCALE
            b1n[il * H:(il + 1) * H, t] = -b1[i] * WSCALE
            sel[32 * c + 2 * v + il, i] = 1.0
    p = np.arange(128)
    msk = (p[:, None] // NINS == p[None, :] // NINS).astype(BF_NP)
    return dict(w1e8=w1e8, w1eb=w1eb, w1o=w1o, w2t=w2t.astype(BF_NP),
                b1n=b1n, msk=msk, sel=sel.astype(BF_NP))


def get_nc():
    global _CACHED_NC
    if _CACHED_NC is None:
        _CACHED_NC = build_nc()
    return _CACHED_NC


def make_in_maps(inputs, W1, b1, W2):
    consts = host_prep(W1, b1, W2)
    inputs = np.asarray(inputs, dtype=np.float32)
    in_maps = []
    for core in range(NCORES):
        shard = np.ascontiguousarray(
            inputs[core * BC:(core + 1) * BC].reshape(R, D))
        m = dict(consts)
        # natural rows, blocked: xn[p, blk, :] = x[blk*128+p, :]
        m["xn"] = np.ascontiguousarray(
            shard.reshape(NBLK, 128, D).transpose(1, 0, 2)).astype(BF_NP)
        st = shard.T  # [768, 4096]
        # fp8 ev part, chunk-major with (kp, ko) interleave
        m["xt8"] = np.ascontiguousarray(
            st[:512].reshape(KP8, 2, 128, NCH, CHUNK)
            .transpose(2, 3, 0, 1, 4)).astype(FP8_NP)
        # bf16 ev part
        m["xtb"] = np.ascontiguousarray(
            st[512:].reshape(KB16, 128, NCH, CHUNK)
            .transpose(1, 2, 0, 3)).astype(BF_NP)
        # j-major transpose for own: rows (j, b); xt2[p, t, k, col]
        x2 = shard.reshape(BC, NINS, D).transpose(1, 0, 2).reshape(R, D)
        m["xt2"] = np.ascontiguousarray(
            x2.T.reshape(KT, 128, MT, CHUNK)
            .transpose(1, 2, 0, 3)).astype(BF_NP)
        in_maps.append(m)
    return in_maps


def kernel(inputs, W1, b1, W2, b2, trace=False):
    """Full-input entry point: shards over 8 cores, returns full output."""
    global LAST_RESULTS
    nc = get_nc()
    in_maps = make_in_maps(inputs, W1, b1, W2)
    res = bass_utils.run_bass_kernel_spmd(
        nc, in_maps, core_ids=list(range(NCORES)), trace=trace)
    LAST_RESULTS = res
    out = np.concatenate(
        [np.asarray(r["out"]).astype(np.float32).transpose(1, 0, 2)
         .reshape(BC, NINS, D)
         for r in res.results],
        axis=0)
    return out


if __name__ == "__main__":
    if "--build" in sys.argv:
        get_nc()
        print("build OK")
